# revision 1
# baseline (speedup 1.0000x reference)
"""ConvAttention Trainium2 kernel.

Per-core (data-parallel over batch, 8 cores, 1 image each):
  q/k/v = depthwise 3x3 conv over x [56,56,64] (+bias), then full
  attention over N=3136 tokens with softmax(q.k * 8), then ctx @ Wp + bp.

Layout strategy:
  - x is transposed on-chip (PE transposes) into a zero-padded [C=64, 58*58]
    "image" so each conv tap is a strided SBUF read.
  - convs run on the PE as diagonal-weight matmuls; k and v share one
    matmul (M=128: k rows 0-63, v rows 64-127), q separate (M=64).
  - scores are computed transposed: s^T[k_token, q_token] so softmax's
    k-reduction can be done by the AV matmul itself (ones column in v).
  - exp runs on ACT straight out of PSUM with scale=8.0 (no max pass --
    scores*8 max out around +-50, far from fp32 overflow).
  - all matmuls use float32r (~12-bit mantissa, full PE speed at N>=256).
  - normalization (1/rowsum) and +bp are folded after the Wp projection;
    final PE transpose brings the result back to [token, embed].
"""

import sys

import numpy as np

if "/opt/trn_rl_repo" not in sys.path:
    sys.path.insert(0, "/opt/trn_rl_repo")

H = 56
W = 56
C = 64
E = 64
N = H * W               # 3136 tokens
HP = H + 2              # padded
WP = W + 2
NPIX = HP * WP          # 3364
NQ = 448                # q-tile (8 spatial rows)
NQT = N // NQ           # 7
KC = 128                # k-chunk (partition dim of s^T tiles)
NKC = (N + KC - 1) // KC  # 25 (last chunk is 64 real tokens)
NPAD = NKC * KC         # 3200 (k padded with zeros)
TCH = 112               # x-transpose chunk = 2 spatial rows
NCORES = 8

_CACHE = {}


def _build(level=99):
    # level: 1=setup 2=+x-transpose 3=+kv-conv/v_nat 4=+q-conv 5=+attn 99=full
    import concourse.bacc as bacc
    import concourse.tile as tile
    from concourse import mybir
    from concourse.masks import make_identity

    F32 = mybir.dt.float32
    F32R = mybir.dt.float32r
    AF = mybir.ActivationFunctionType

    nc = bacc.Bacc(None, target_bir_lowering=False, debug=False)

    x_d = nc.dram_tensor("x", [N, C], F32, kind="ExternalInput")
    wq_d = nc.dram_tensor("wq", [9, C], F32, kind="ExternalInput")
    bq_d = nc.dram_tensor("bq", [C], F32, kind="ExternalInput")
    wk_d = nc.dram_tensor("wk", [9, C], F32, kind="ExternalInput")
    bk_d = nc.dram_tensor("bk", [C], F32, kind="ExternalInput")
    wv_d = nc.dram_tensor("wv", [9, C], F32, kind="ExternalInput")
    bv_d = nc.dram_tensor("bv", [C], F32, kind="ExternalInput")
    Wp_d = nc.dram_tensor("Wp", [C, E], F32, kind="ExternalInput")
    bp_d = nc.dram_tensor("bp", [E], F32, kind="ExternalInput")
    out_d = nc.dram_tensor("out", [N, E], F32, kind="ExternalOutput")

    with tile.TileContext(nc) as tc:
        with tc.tile_pool(name="const", bufs=1) as const, \
             tc.tile_pool(name="big", bufs=1) as big:
            # identity: build in f32 (memset/affine_select can't write f32r),
            # then round-copy to f32r for use with f32r transposes
            ident_f = const.tile([128, 128], F32)
            make_identity(nc, ident_f[:])
            ident = const.tile([128, 128], F32R)
            nc.vector.tensor_copy(ident[:], ident_f[:])
            zsc = const.tile([128, 128], F32)
            nc.vector.memset(zsc[:], 0.0)
            ones25 = const.tile([128, NKC], F32)
            nc.vector.memset(ones25[:], 1.0)

            # per-channel weights/biases as [partition, tap] scalars
            wqT = const.tile([C, 9], F32)
            nc.sync.dma_start(wqT[:], wq_d[:].transpose([1, 0]))
            wkT = const.tile([C, 9], F32)
            nc.sync.dma_start(wkT[:], wk_d[:].transpose([1, 0]))
            wvT = const.tile([C, 9], F32)
            nc.sync.dma_start(wvT[:], wv_d[:].transpose([1, 0]))
            bqT = const.tile([C, 1], F32)
            nc.sync.dma_start(bqT[:], bq_d[:].unsqueeze(1))
            bkvT = const.tile([128, 1], F32)
            nc.sync.dma_start(bkvT[0:C, :], bk_d[:].unsqueeze(1))
            nc.sync.dma_start(bkvT[C:128, :], bv_d[:].unsqueeze(1))

            # conv lhsT blocks: diagonal(w_tap)
            qw = const.tile([C, 9, C], F32R)
            kvw = const.tile([C, 9, 128], F32R)
            for t in range(9):
                nc.vector.tensor_scalar_mul(qw[:, t, :], ident[0:C, 0:C], wqT[:, t:t + 1])
                nc.vector.tensor_scalar_mul(kvw[:, t, 0:C], ident[0:C, 0:C], wkT[:, t:t + 1])
                nc.vector.tensor_scalar_mul(kvw[:, t, C:128], ident[0:C, 0:C], wvT[:, t:t + 1])

            # projection weights with bias row: [Wp; bp] (K=65)
            wp_aug = const.tile([C + 1, E], F32R)
            nc.gpsimd.dma_start(wp_aug[0:C, :], Wp_d[:])
            nc.gpsimd.dma_start(wp_aug[C:C + 1, :], bp_d[:].unsqueeze(0))

            # stage x: [112, 28, 64] via 4 chunked HWDGE loads so the first
            # transpose starts early (cast to f32r happens in the PSUM->SBUF copy)
            xstage = big.tile([TCH, N // TCH, C], F32)
            xsrc = x_d[:].rearrange("(r p) c -> p r c", p=TCH)
            for dc in range(4):
                nc.sync.dma_start(xstage[:, dc * 7:(dc + 1) * 7, :],
                                  xsrc[:, dc * 7:(dc + 1) * 7, :])

            # big persistent tensors
            xpT = big.tile([C, HP, WP], F32R)          # padded transposed image
            qT = big.tile([C, N], F32R)                # q^T  [c, token]
            kvT = big.tile([128, NPAD], F32R)          # rows 0-63 k^T, 64-127 v^T
            v_nat = big.tile([128, NKC, C + 1], F32R)  # [token%128, chunk, c|ones]
            # final-stage transpose staging: rows 0-63 proj^T, row 64 rowsum,
            # rows 65-95 zero padding (PE transpose needs K % 32 == 0)
            t2a = big.tile([96, NQ], F32)
            t2b = big.tile([96, NQ], F32)
            nc.vector.memset(t2a[64:96, :], 0.0)
            nc.vector.memset(t2b[64:96, :], 0.0)

            # zero-fill f32r regions via f32->f32r copies (memset can't emit f32r):
            # xpT border rows/cols, kvT's k-token padding, and the garbage
            # upper half of v_nat's last (64-token) chunk; ones column for
            # the rowsum trick.
            nc.vector.tensor_copy(xpT[:, 0, :], zsc[0:C, 0:WP])
            nc.vector.tensor_copy(xpT[:, HP - 1, :], zsc[0:C, 0:WP])
            nc.vector.tensor_copy(xpT[:, :, 0:1], zsc[0:C, 0:HP].unsqueeze(2))
            nc.vector.tensor_copy(xpT[:, :, WP - 1:WP], zsc[0:C, 0:HP].unsqueeze(2))
            nc.vector.tensor_copy(kvT[:, N:NPAD], zsc[:, 0:NPAD - N])
            nc.vector.tensor_copy(v_nat[:, :, C], ones25[:])
            nc.vector.tensor_copy(v_nat[C:128, NKC - 1, :], zsc[C:128, 0:C + 1])

            with tc.tile_pool(name="ps1", bufs=2, space="PSUM") as ps1, \
                 tc.tile_pool(name="ps1b", bufs=4, space="PSUM") as ps1b, \
                 tc.tile_pool(name="ps1c", bufs=2, space="PSUM") as ps1c:
                # x -> xpT (PE transpose 2 spatial rows at a time); copies
                # alternate between DVE and ACT to halve the copy wall-time
                for r in range(N // TCH if level >= 2 else 0):
                    pt = ps1b.tile([C, TCH], F32, tag="tp")
                    nc.tensor.transpose(pt[:], xstage[:, r, :], ident_f[0:TCH, 0:TCH])
                    dst = xpT[:, 1 + 2 * r:3 + 2 * r, 1:1 + W]
                    src = pt[:].rearrange("c (h w) -> c h w", w=W)
                    if r % 2 == 0:
                        nc.vector.tensor_copy(dst, src)
                    else:
                        nc.scalar.copy(dst, src)

                # k+v convs (paired, M=128)
                for ct in range(NQT if level >= 3 else 0):
                    pkv = ps1.tile([128, NQ], F32, tag="cv")
                    for t in range(9):
                        i, j = t // 3, t % 3
                        nc.tensor.matmul(
                            pkv[:], kvw[:, t, :],
                            xpT[:, ct * 8 + i:ct * 8 + i + 8, j:j + W],
                            start=(t == 0), stop=(t == 8))
                    nc.vector.tensor_scalar_add(
                        kvT[:, ct * NQ:(ct + 1) * NQ], pkv[:], bkvT[:, 0:1])

                # v^T -> v_nat (PE transpose, 128-token chunks)
                for kc in range(NKC if level >= 3 else 0):
                    cw = min(KC, N - kc * KC)
                    tp = ps1c.tile([128, C], F32R, tag="tp2")
                    nc.tensor.transpose(
                        tp[0:cw, :], kvT[C:128, kc * KC:kc * KC + cw],
                        ident[C:128, C:128])
                    if kc % 2 == 0:
                        nc.vector.tensor_copy(v_nat[0:cw, kc, 0:C], tp[0:cw, :])
                    else:
                        nc.scalar.copy(v_nat[0:cw, kc, 0:C], tp[0:cw, :])

            with tc.tile_pool(name="ps2", bufs=1, space="PSUM") as ps2, \
                 tc.tile_pool(name="psS", bufs=2, space="PSUM") as psS, \
                 tc.tile_pool(name="psC", bufs=1, space="PSUM") as psC, \
                 tc.tile_pool(name="psF", bufs=2, space="PSUM") as psF, \
                 tc.tile_pool(name="sbA", bufs=3) as sbA, \
                 tc.tile_pool(name="sbB", bufs=2) as sbB:
                for qt in range(NQT if level >= 4 else 0):
                    q0 = qt * NQ
                    # q conv for this tile
                    pq = ps2.tile([C, NQ], F32, tag="qcv")
                    for t in range(9):
                        i, j = t // 3, t % 3
                        nc.tensor.matmul(
                            pq[:], qw[:, t, :],
                            xpT[:, qt * 8 + i:qt * 8 + i + 8, j:j + W],
                            start=(t == 0), stop=(t == 8))
                    nc.vector.tensor_scalar_add(
                        qT[:, q0:q0 + NQ], pq[:], bqT[:, 0:1])

                    if level < 5:
                        continue
                    # attention: s^T chunks -> exp -> AV accumulate
                    pctx = psC.tile([C + 1, NQ], F32, tag="ctx")
                    for b in range((NKC + 1) // 2):
                        nb = min(2, NKC - b * 2)
                        # last chunk holds only 64 real k-tokens; shrink it
                        pw = 64 if (b * 2 + nb) == NKC else 128
                        ps_s = psS.tile([128, 2, 512], F32, tag="s")
                        pT = sbA.tile([128, 2, NQ], F32R, tag="p")
                        for jj in range(nb):
                            kc = b * 2 + jj
                            cw = 64 if kc == NKC - 1 else 128
                            nc.tensor.matmul(
                                ps_s[0:cw, jj, 0:NQ],
                                kvT[0:C, kc * KC:kc * KC + cw],
                                qT[:, q0:q0 + NQ],
                                start=True, stop=True)
                        nc.scalar.activation(
                            pT[0:pw, 0:nb, :], ps_s[0:pw, 0:nb, 0:NQ],
                            AF.Exp, scale=8.0)
                        for jj in range(nb):
                            kc = b * 2 + jj
                            cw = 64 if kc == NKC - 1 else 128
                            nc.tensor.matmul(
                                pctx[:], v_nat[0:cw, kc, :], pT[0:cw, jj, :],
                                start=(kc == 0), stop=(kc == NKC - 1))

                    if level < 6:
                        continue
                    ctxT = sbB.tile([C + 1, NQ], F32R, tag="ctxT")
                    nc.vector.tensor_copy(ctxT[:], pctx[:])

                    pp2 = psF.tile([128, NQ], F32, tag="fin")
                    nc.tensor.matmul(pp2[0:E, :], wp_aug[:], ctxT[:],
                                     start=True, stop=True)
                    t2 = t2a if qt % 2 == 0 else t2b
                    nc.vector.tensor_copy(t2[0:E, :], pp2[0:E, :])
                    nc.vector.tensor_copy(t2[C:C + 1, :], ctxT[C:C + 1, :])

                    fin = sbB.tile([TCH, 4, E], mybir.dt.float32, tag="fin4")
                    for c4 in range(4):
                        pf = psF.tile([128, 96], F32, tag="fin")
                        nc.tensor.transpose(
                            pf[0:TCH, :], t2[:, c4 * TCH:(c4 + 1) * TCH],
                            ident_f[0:96, 0:96])
                        inv = sbB.tile([TCH, 1], mybir.dt.float32, tag="inv")
                        nc.vector.reciprocal(inv[:], pf[0:TCH, C:C + 1])
                        nc.vector.tensor_scalar_mul(
                            fin[:, c4, :], pf[0:TCH, 0:E], inv[:, 0:1])
                    nc.sync.dma_start(
                        out_d[q0:q0 + NQ, :].rearrange("(c p) e -> p c e", p=TCH),
                        fin[:])

    nc.compile()
    return nc


def _get_nc():
    if "nc" not in _CACHE:
        _CACHE["nc"] = _build()
    return _CACHE["nc"]


def kernel(x, wq, bq, wk, bk, wv, bv, Wp, bp):
    from concourse.bass_utils import run_bass_kernel_spmd

    nc = _get_nc()
    x = np.ascontiguousarray(np.asarray(x, dtype=np.float32))
    shared = {
        "wq": np.ascontiguousarray(np.asarray(wq, np.float32).reshape(9, C)),
        "bq": np.ascontiguousarray(np.asarray(bq, np.float32)),
        "wk": np.ascontiguousarray(np.asarray(wk, np.float32).reshape(9, C)),
        "bk": np.ascontiguousarray(np.asarray(bk, np.float32)),
        "wv": np.ascontiguousarray(np.asarray(wv, np.float32).reshape(9, C)),
        "bv": np.ascontiguousarray(np.asarray(bv, np.float32)),
        "Wp": np.ascontiguousarray(np.asarray(Wp, np.float32)),
        "bp": np.ascontiguousarray(np.asarray(bp, np.float32)),
    }
    in_maps = [dict(shared, x=x[i].reshape(N, C)) for i in range(NCORES)]
    res = run_bass_kernel_spmd(nc, in_maps, core_ids=list(range(NCORES)))
    out = np.stack([res.results[i]["out"].reshape(H, W, E) for i in range(NCORES)])
    return out



# revision 9
# speedup vs baseline: 1.1004x; 1.1004x over previous
"""ConvAttention Trainium2 kernel (v2).

Per-core (data-parallel over batch, 8 cores, 1 image each):
  q/k/v = depthwise 3x3 conv over x [56,56,64], then full attention over
  N=3136 tokens with softmax(q.k * 8), then ctx @ Wp + bp.

v2 layout strategy (vs v1):
  - Wp is folded into the v-conv (lhsT blocks diag(wv_t) @ Wp), so the AV
    matmul directly produces the projected output; bv/bp fold into a single
    bias b' = bv@Wp + bp added to v'' (exact via the rowsum trick); bk is
    dropped entirely (constant along the softmax axis -> cancels).
  - AV is restructured to out[qtok<=128, e] with lhsT = p^T chunks: output
    lands in natural [token, embed] orientation, so no final transposes,
    no projection matmul, and normalization is a per-partition scalar op.
  - p and v'' are bf16 (emulated end-to-end rel err ~2.8e-3); q,k stay
    f32r (fp8/bf16 scores fail the 2e-2 gate due to the x8 logit scale).
  - exp runs on ACT from PSUM in 3-chunk groups ([128,3,448] per instr)
    to amortize the per-instruction SBUF-access overhead; ACT does nothing
    else during the attention phase (it is the wall at ~75us busy).
  - QK of group g+1 is emitted before AV of group g so the in-order PE
    stream never stalls ACT.
"""

import sys

import numpy as np

if "/opt/trn_rl_repo" not in sys.path:
    sys.path.insert(0, "/opt/trn_rl_repo")

H = 56
W = 56
C = 64
E = 64
N = H * W               # 3136 tokens
HP = H + 2              # padded
WP = W + 2
NQ = 448                # q-tile (8 spatial rows)
NQT = N // NQ           # 7
KC = 128                # k-chunk (partition dim of s^T tiles)
NKC = (N + KC - 1) // KC  # 25 (last chunk is 64 real tokens)
NPAD = NKC * KC         # 3200 (k padded with zeros)
TCH = 112               # x-transpose chunk = 2 spatial rows
G = 3                   # k-chunks per exp instruction
NCORES = 8

_CACHE = {}


def _build(level=99):
    import concourse.bacc as bacc
    import concourse.tile as tile
    from concourse import mybir
    from concourse.masks import make_identity

    F32 = mybir.dt.float32
    F32R = mybir.dt.float32r
    BF16 = mybir.dt.bfloat16
    AF = mybir.ActivationFunctionType

    nc = bacc.Bacc(None, target_bir_lowering=False, debug=False)

    x_d = nc.dram_tensor("x", [N, C], F32, kind="ExternalInput")
    wq_d = nc.dram_tensor("wq", [9, C], F32, kind="ExternalInput")
    bq_d = nc.dram_tensor("bq", [C], F32, kind="ExternalInput")
    wk_d = nc.dram_tensor("wk", [9, C], F32, kind="ExternalInput")
    wv_d = nc.dram_tensor("wv", [9, C], F32, kind="ExternalInput")
    bv_d = nc.dram_tensor("bv", [C], F32, kind="ExternalInput")
    Wp_d = nc.dram_tensor("Wp", [C, E], F32, kind="ExternalInput")
    bp_d = nc.dram_tensor("bp", [E], F32, kind="ExternalInput")
    out_d = nc.dram_tensor("out", [N, E], F32, kind="ExternalOutput")

    with tile.TileContext(nc) as tc:
        with tc.tile_pool(name="const", bufs=1) as const, \
             tc.tile_pool(name="big", bufs=1) as big:
            ident_f = const.tile([128, 128], F32)
            make_identity(nc, ident_f[:])
            ident = const.tile([128, 128], F32R)
            nc.vector.tensor_copy(ident[:], ident_f[:])
            ident_b = const.tile([128, 128], BF16)
            nc.vector.tensor_copy(ident_b[:], ident_f[:])
            zsc = const.tile([128, 128], F32)
            nc.vector.memset(zsc[:], 0.0)
            ones_f = const.tile([128, NKC], F32)
            nc.vector.memset(ones_f[:], 1.0)

            # per-channel weights/biases as [partition, tap] scalars
            wqT = const.tile([C, 9], F32)
            nc.sync.dma_start(wqT[:], wq_d[:].transpose([1, 0]))
            wkT = const.tile([C, 9], F32)
            nc.sync.dma_start(wkT[:], wk_d[:].transpose([1, 0]))
            wvT = const.tile([C, 9], F32)
            nc.sync.dma_start(wvT[:], wv_d[:].transpose([1, 0]))
            bqT = const.tile([C, 1], F32)
            nc.sync.dma_start(bqT[:], bq_d[:].unsqueeze(1))
            bvT = const.tile([C, 1], F32)
            nc.gpsimd.dma_start(bvT[:], bv_d[:].unsqueeze(1))
            bpT = const.tile([C, 1], F32)
            nc.gpsimd.dma_start(bpT[:], bp_d[:].unsqueeze(1))
            Wp_f = const.tile([C, E], F32)
            nc.gpsimd.dma_start(Wp_f[:], Wp_d[:])
            Wp_r = const.tile([C, E], F32R)
            nc.vector.tensor_copy(Wp_r[:], Wp_f[:])
            Wp_b = const.tile([C, E], BF16)
            nc.vector.tensor_copy(Wp_b[:], Wp_f[:])
            bv_b = const.tile([C, 1], BF16)
            nc.vector.tensor_copy(bv_b[:], bvT[:])

            # conv lhsT blocks: q diag(wq_t); kv: cols 0-63 diag(wk_t),
            # cols 64-127 diag(wv_t) @ Wp (projection folded in)
            qw = const.tile([C, 9, C], F32R)
            kvw = const.tile([C, 9, 128], F32R)
            for t in range(9):
                nc.vector.tensor_scalar_mul(qw[:, t, :], ident[0:C, 0:C], wqT[:, t:t + 1])
                nc.vector.tensor_scalar_mul(kvw[:, t, 0:C], ident[0:C, 0:C], wkT[:, t:t + 1])
                nc.vector.tensor_scalar_mul(kvw[:, t, C:128], Wp_r[:], wvT[:, t:t + 1])

            # stage x: [112, 28, 64] via 4 chunked HWDGE loads so the first
            # transpose starts early
            xstage = big.tile([TCH, N // TCH, C], F32)
            xsrc = x_d[:].rearrange("(r p) c -> p r c", p=TCH)
            for dc in range(4):
                nc.sync.dma_start(xstage[:, dc * 7:(dc + 1) * 7, :],
                                  xsrc[:, dc * 7:(dc + 1) * 7, :])

            # big persistent tensors
            xpT = big.tile([C, HP, WP], F32R)      # padded transposed image
            qT = big.tile([C, N], F32R)            # q^T  [c, token]
            kT = big.tile([C, NPAD], F32R)         # k^T  [c, token], zero pad
            vT = big.tile([128, N], BF16)          # v''^T on partitions 64-127
            v_nat = big.tile([128, NKC, C + 1], BF16)  # [tok%128, chunk, e|ones]
            b1 = big.tile([128, 1], F32)           # b' = bv@Wp + bp (parts 64+)
            b1sb = big.tile([C, 1], F32)           # b' staged at parts 0-63

            # zero-fill f32r regions (memset can't emit f32r)
            nc.vector.tensor_copy(xpT[:, 0, :], zsc[0:C, 0:WP])
            nc.vector.tensor_copy(xpT[:, HP - 1, :], zsc[0:C, 0:WP])
            nc.vector.tensor_copy(xpT[:, :, 0:1], zsc[0:C, 0:HP].unsqueeze(2))
            nc.vector.tensor_copy(xpT[:, :, WP - 1:WP], zsc[0:C, 0:HP].unsqueeze(2))
            nc.vector.tensor_copy(kT[:, N:NPAD], zsc[0:C, 0:NPAD - N])
            # ones column for the rowsum trick
            nc.vector.tensor_copy(v_nat[:, :, C], ones_f[:])

            with tc.tile_pool(name="ps1", bufs=2, space="PSUM") as ps1, \
                 tc.tile_pool(name="ps1b", bufs=3, space="PSUM") as ps1b, \
                 tc.tile_pool(name="ps1c", bufs=2, space="PSUM") as ps1c, \
                 tc.tile_pool(name="psB", bufs=1, space="PSUM") as psB:
                # b' = bv @ Wp + bp, computed at partitions 0-63 (bf16 matmul
                # dodges fp32r ISA restrictions), then DMA-shifted to 64-127
                pb = psB.tile([C, 1], F32, tag="b1")
                nc.tensor.matmul(pb[:], Wp_b[:], bv_b[:], start=True, stop=True)
                nc.vector.tensor_tensor(b1sb[:], pb[:], bpT[:],
                                        mybir.AluOpType.add)
                nc.sync.dma_start(b1[C:128, :], b1sb[:])

                # x -> xpT (PE transposes); copies alternate DVE / ACT
                for r in range(N // TCH if level >= 2 else 0):
                    pt = ps1b.tile([C, TCH], F32, tag="tp")
                    nc.tensor.transpose(pt[:], xstage[:, r, :], ident_f[0:TCH, 0:TCH])
                    dst = xpT[:, 1 + 2 * r:3 + 2 * r, 1:1 + W]
                    src = pt[:].rearrange("c (h w) -> c h w", w=W)
                    if r % 2 == 0:
                        nc.vector.tensor_copy(dst, src)
                    else:
                        nc.scalar.copy(dst, src)

                # k+v'' convs (paired, M=128): k rows 0-63, v'' rows 64-127
                for ct in range(NQT if level >= 3 else 0):
                    pkv = ps1.tile([128, NQ], F32, tag="cv")
                    for t in range(9):
                        i, j = t // 3, t % 3
                        nc.tensor.matmul(
                            pkv[:], kvw[:, t, :],
                            xpT[:, ct * 8 + i:ct * 8 + i + 8, j:j + W],
                            start=(t == 0), stop=(t == 8))
                    nc.vector.tensor_copy(kT[:, ct * NQ:(ct + 1) * NQ],
                                          pkv[0:C, :])
                    nc.vector.tensor_scalar_add(
                        vT[C:128, ct * NQ:(ct + 1) * NQ], pkv[C:128, :],
                        b1[C:128, 0:1])

                # v''^T -> v_nat (PE transpose, 128-token chunks)
                for kc in range(NKC if level >= 3 else 0):
                    cw = min(KC, N - kc * KC)
                    tp = ps1c.tile([128, C], BF16, tag="tp2")
                    nc.tensor.transpose(
                        tp[0:cw, :], vT[C:128, kc * KC:kc * KC + cw],
                        ident_b[C:128, C:128])
                    nc.vector.tensor_copy(v_nat[0:cw, kc, 0:C], tp[0:cw, :])

            ngrp = (NKC + G - 1) // G  # 9: 8 groups of 3 + 1 of 1
            # taps of tile qt+1's q-conv spread across tile qt's groups so
            # the 1.7us conv never bubbles the exp pipeline
            tap_sched = {0: (0, 1), 1: (2,), 2: (3,), 3: (4,), 4: (5,),
                         5: (6,), 6: (7, 8)}
            # one PSUM bank (= one zero region) only fits ONE pending matmul
            # accumulation group, so the 4 q-subtiles of a tile accumulate
            # their ctx sequentially; p^T for a whole tile stays resident in
            # sbP (bufs = 2 tiles x 9 groups) and AV of tile t-1 is flushed
            # in half-sub batches between tile t's QK groups.
            with tc.tile_pool(name="ps2", bufs=1, space="PSUM") as ps2, \
                 tc.tile_pool(name="psS", bufs=2, space="PSUM") as psS, \
                 tc.tile_pool(name="psC", bufs=1, space="PSUM") as psC, \
                 tc.tile_pool(name="sbP", bufs=2 * ngrp) as sbP, \
                 tc.tile_pool(name="sbO", bufs=4) as sbO, \
                 tc.tile_pool(name="sbI", bufs=4) as sbI:

                def emit_qconv_taps(pq, qt, taps):
                    for t in taps:
                        i, j = t // 3, t % 3
                        nc.tensor.matmul(
                            pq[:], qw[:, t, :],
                            xpT[:, qt * 8 + i:qt * 8 + i + 8, j:j + W],
                            start=(t == 0), stop=(t == 8))

                def emit_qcopy(pq, qt):
                    nc.vector.tensor_scalar_add(
                        qT[:, qt * NQ:(qt + 1) * NQ], pq[:], bqT[:, 0:1])

                state = {"pctx": None}

                def emit_av_batch(pT_tiles, s, half):
                    s0 = s * 128
                    sw = min(128, NQ - s0)
                    if half == 0:
                        state["pctx"] = psC.tile([128, C + 1], F32,
                                                 name="pctx", tag="ctx")
                    pctx = state["pctx"]
                    chunks = range(0, 13) if half == 0 else range(13, NKC)
                    for kc in chunks:
                        g, j = kc // G, kc % G
                        cw = 64 if kc == NKC - 1 else 128
                        nc.tensor.matmul(
                            pctx[0:sw, :],
                            pT_tiles[g][0:cw, j, s0:s0 + sw],
                            v_nat[0:cw, kc, :],
                            start=(kc == 0), stop=(kc == NKC - 1))

                def emit_norm_sub(qt, s):
                    pctx = state["pctx"]
                    s0 = s * 128
                    sw = min(128, NQ - s0)
                    inv = sbI.tile([128, 1], F32, tag="inv")
                    nc.vector.reciprocal(inv[0:sw, :], pctx[0:sw, C:C + 1])
                    osb = sbO.tile([128, E], F32, tag="out")
                    nc.vector.tensor_scalar_mul(
                        osb[0:sw, :], pctx[0:sw, 0:C], inv[0:sw, 0:1])
                    nc.sync.dma_start(
                        out_d[qt * NQ + s0:qt * NQ + s0 + sw, :], osb[0:sw, :])

                def flush_prev(prev, g):
                    # schedule: g0/g1 -> sub0 halves, g1 also norm(sub0), ...
                    if prev is None:
                        return
                    qt_prev, pT_tiles = prev
                    if g < 8:
                        emit_av_batch(pT_tiles, g // 2, g % 2)
                        if g % 2 == 1:
                            emit_norm_sub(qt_prev, g // 2)

                if level >= 4:
                    pq = ps2.tile([C, NQ], F32, tag="qcv")
                    emit_qconv_taps(pq, 0, range(9))
                    emit_qcopy(pq, 0)

                prev = None
                for qt in range(NQT if level >= 5 else 0):
                    q0 = qt * NQ
                    pq_next = None
                    if qt + 1 < NQT:
                        pq_next = ps2.tile([C, NQ], F32, tag="qcv")
                    pT_tiles = []
                    for g in range(ngrp):
                        nb = min(G, NKC - g * G)
                        pw = 64 if (g * G + nb) == NKC else 128
                        ps_s = psS.tile([128, G, 512], F32, tag="s")
                        for j in range(nb):
                            kc = g * G + j
                            cw = 64 if kc == NKC - 1 else 128
                            nc.tensor.matmul(
                                ps_s[0:cw, j, 0:NQ],
                                kT[:, kc * KC:kc * KC + cw],
                                qT[:, q0:q0 + NQ],
                                start=True, stop=True)
                        flush_prev(prev, g)
                        if pq_next is not None and g in tap_sched:
                            emit_qconv_taps(pq_next, qt + 1, tap_sched[g])
                        if pq_next is not None and g == 7:
                            emit_qcopy(pq_next, qt + 1)
                        pTt = sbP.tile([128, G, NQ], BF16, tag="p")
                        nc.scalar.activation(
                            pTt[0:pw, 0:nb, :], ps_s[0:pw, 0:nb, 0:NQ],
                            AF.Exp, scale=8.0)
                        pT_tiles.append(pTt)
                    if level >= 6:
                        prev = (qt, pT_tiles)

                if prev is not None:
                    for g in range(8):
                        flush_prev(prev, g)

    nc.compile()
    return nc


def _get_nc():
    if "nc" not in _CACHE:
        _CACHE["nc"] = _build()
    return _CACHE["nc"]


def kernel(x, wq, bq, wk, bk, wv, bv, Wp, bp):
    from concourse.bass_utils import run_bass_kernel_spmd

    nc = _get_nc()
    x = np.ascontiguousarray(np.asarray(x, dtype=np.float32))
    shared = {
        "wq": np.ascontiguousarray(np.asarray(wq, np.float32).reshape(9, C)),
        "bq": np.ascontiguousarray(np.asarray(bq, np.float32)),
        "wk": np.ascontiguousarray(np.asarray(wk, np.float32).reshape(9, C)),
        "wv": np.ascontiguousarray(np.asarray(wv, np.float32).reshape(9, C)),
        "bv": np.ascontiguousarray(np.asarray(bv, np.float32)),
        "Wp": np.ascontiguousarray(np.asarray(Wp, np.float32)),
        "bp": np.ascontiguousarray(np.asarray(bp, np.float32)),
    }
    in_maps = [dict(shared, x=x[i].reshape(N, C)) for i in range(NCORES)]
    res = run_bass_kernel_spmd(nc, in_maps, core_ids=list(range(NCORES)))
    out = np.stack([res.results[i]["out"].reshape(H, W, E) for i in range(NCORES)])
    return out


# revision 17
# speedup vs baseline: 1.1099x; 1.0087x over previous
"""ConvAttention Trainium2 kernel (v4).

Per-core (data-parallel over batch, 8 cores, 1 image each):
  q/k/v = depthwise 3x3 conv over x [56,56,64], then full attention over
  N=3136 tokens with softmax(q.k * 8), then ctx @ Wp + bp.

Layout strategy:
  - Wp folded into the v-conv (lhsT blocks diag(wv_t) @ Wp): AV directly
    produces the projected output. bv/bp fold into b' = bv@Wp + bp added to
    v'' (exact via the rowsum trick); bk is dropped (constant along the
    softmax axis -> cancels exactly).
  - Convs are tap-stacked K=128 matmuls: partitions 64-127 of the padded
    transposed image hold a one-row-shifted copy (xpT) / one-col-shifted
    copy (xp3), built by SBUF->SBUF DMAs, so taps (0,j)+(1,j) and
    (2,0)+(2,1) pair into single matmuls: 5 matmuls per conv tile vs 9.
  - AV is out[qtok<=128, e] with lhsT = p^T chunks: natural [token, embed]
    output, no final transposes, no projection matmul; normalization is a
    per-partition reciprocal+mul. p/v'' are bf16 (rel err ~3e-3); q,k stay
    f32r (bf16/fp8 scores fail the 2e-2 gate via the x8 logit scale).
  - exp is the wall (~78us ACT busy): it runs from PSUM in alternating
    3-chunk/2-chunk groups (score pools of 3+2 banks) to amortize ACT
    access overhead while double-buffering QK against exp.
  - Everything else hides under exp: kv-convs + v_nat transposes interleave
    into attention tile 0's groups, the next tile's q-conv slots spread one
    per group, AV of tile t-1 flushes in half-sub batches between tile t's
    QK groups, and PSUM accumulation groups each keep an exclusive 2KB
    zero region (psSa 3 + psSb 2 + ps2 1 + psT 1 + psC 1 = 8 banks).
"""

import sys

import numpy as np

if "/opt/trn_rl_repo" not in sys.path:
    sys.path.insert(0, "/opt/trn_rl_repo")

H = 56
W = 56
C = 64
E = 64
N = H * W               # 3136 tokens
HP = H + 2              # padded
WP = W + 2
NQ = 448                # q-tile (8 spatial rows)
NQT = N // NQ           # 7
KC = 128                # k-chunk (partition dim of s^T tiles)
NKC = (N + KC - 1) // KC  # 25 (last chunk is 64 real tokens)
NPAD = NKC * KC         # 3200 (k padded with zeros)
TCH = 112               # x-transpose chunk = 2 spatial rows
NTCH = N // TCH         # 28
NCORES = 8

# exp chunk-groups per tile: alternating 3/2 so the two score pools fit in
# 5 PSUM banks total while still double-buffering QK against exp
GRP_SIZES = [3, 2, 3, 2, 3, 2, 3, 2, 3, 2]
GRP_OFF = [0, 3, 5, 8, 10, 13, 15, 18, 20, 23]
NGRP = len(GRP_SIZES)
# kv-conv tile that must be complete before QK of group g (any q-tile)
KV_NEED = [min(((GRP_OFF[g] + GRP_SIZES[g]) * KC - 1) // NQ, NQT - 1)
           for g in range(NGRP)]
# stacked conv slots: (lower tap, upper tap or None); taps t = 3*i + j
CONV_SLOTS = [(0, 3), (1, 4), (2, 5), (6, 7), (8, None)]
# x-transpose chunk whose copy must land before kv-conv tile ct can run
# (the row/col-shifted upper halves arrive via bulk DMAs gated on chunks
# 6/13/20/27; tile ct reads shifted rows up to 8*ct+9)
KV_CHUNK_NEED = [min(7 * ((8 * ct + 9) // 14) + 7, NTCH) for ct in range(NQT)]

_CACHE = {}


def _build(level=99):
    import concourse.bacc as bacc
    import concourse.tile as tile
    from concourse import mybir
    from concourse.masks import make_identity

    F32 = mybir.dt.float32
    F32R = mybir.dt.float32r
    BF16 = mybir.dt.bfloat16
    AF = mybir.ActivationFunctionType

    nc = bacc.Bacc(None, target_bir_lowering=False, debug=False)

    x_d = nc.dram_tensor("x", [N, C], F32, kind="ExternalInput")
    wq_d = nc.dram_tensor("wq", [9, C], F32, kind="ExternalInput")
    bq_d = nc.dram_tensor("bq", [C], F32, kind="ExternalInput")
    wk_d = nc.dram_tensor("wk", [9, C], F32, kind="ExternalInput")
    wv_d = nc.dram_tensor("wv", [9, C], F32, kind="ExternalInput")
    bv_d = nc.dram_tensor("bv", [C], F32, kind="ExternalInput")
    Wp_d = nc.dram_tensor("Wp", [C, E], F32, kind="ExternalInput")
    bp_d = nc.dram_tensor("bp", [E], F32, kind="ExternalInput")
    out_d = nc.dram_tensor("out", [N, E], F32, kind="ExternalOutput")

    with tile.TileContext(nc) as tc:
        with tc.tile_pool(name="const", bufs=1) as const, \
             tc.tile_pool(name="big", bufs=1) as big:
            ident_f = const.tile([128, 128], F32)
            make_identity(nc, ident_f[:])
            ident = const.tile([128, 128], F32R)
            nc.vector.tensor_copy(ident[:], ident_f[:])
            ident_b = const.tile([128, 128], BF16)
            nc.vector.tensor_copy(ident_b[:], ident_f[:])
            zsc = const.tile([128, 128], F32)
            nc.vector.memset(zsc[:], 0.0)
            ones_f = const.tile([128, NKC], F32)
            nc.vector.memset(ones_f[:], 1.0)

            # per-channel conv weights, replicated on both partition halves
            # (the upper half feeds the tap-stacked lhsT rows 64-127)
            wqT = const.tile([128, 9], F32)
            wkT = const.tile([128, 9], F32)
            wvT = const.tile([128, 9], F32)
            for wt, wd in ((wqT, wq_d), (wkT, wk_d), (wvT, wv_d)):
                nc.sync.dma_start(wt[0:C, :], wd[:].transpose([1, 0]))
                nc.sync.dma_start(wt[C:128, :], wd[:].transpose([1, 0]))
            bqT = const.tile([C, 1], F32)
            nc.sync.dma_start(bqT[:], bq_d[:].unsqueeze(1))
            bvT = const.tile([C, 1], F32)
            nc.gpsimd.dma_start(bvT[:], bv_d[:].unsqueeze(1))
            bpT = const.tile([C, 1], F32)
            nc.gpsimd.dma_start(bpT[:], bp_d[:].unsqueeze(1))
            Wp_f = const.tile([128, E], F32)
            nc.gpsimd.dma_start(Wp_f[0:C, :], Wp_d[:])
            nc.gpsimd.dma_start(Wp_f[C:128, :], Wp_d[:])
            Wp_r = const.tile([128, E], F32R)
            nc.vector.tensor_copy(Wp_r[:], Wp_f[:])
            Wp_b = const.tile([C, E], BF16)
            nc.vector.tensor_copy(Wp_b[:], Wp_f[0:C, :])
            bv_b = const.tile([C, 1], BF16)
            nc.vector.tensor_copy(bv_b[:], bvT[:])

            # tap-stacked conv lhsT: st_q [128, 5, 64] (diag(wq)), st_kv
            # [128, 5, 128] (cols 0-63 diag(wk), 64-127 diag(wv) @ Wp)
            st_q = const.tile([128, 5, C], F32R)
            st_kv = const.tile([128, 5, 128], F32R)
            idlo = ident[0:C, 0:C]
            idhi = ident[C:128, C:128]
            for s, (lt, ut) in enumerate(CONV_SLOTS):
                nc.vector.tensor_scalar_mul(st_q[0:C, s, :], idlo, wqT[0:C, lt:lt + 1])
                nc.vector.tensor_scalar_mul(st_kv[0:C, s, 0:C], idlo, wkT[0:C, lt:lt + 1])
                nc.vector.tensor_scalar_mul(st_kv[0:C, s, C:128], Wp_r[0:C, :], wvT[0:C, lt:lt + 1])
                if ut is not None:
                    nc.vector.tensor_scalar_mul(st_q[C:128, s, :], idhi, wqT[C:128, ut:ut + 1])
                    nc.vector.tensor_scalar_mul(st_kv[C:128, s, 0:C], idhi, wkT[C:128, ut:ut + 1])
                    nc.vector.tensor_scalar_mul(st_kv[C:128, s, C:128], Wp_r[C:128, :], wvT[C:128, ut:ut + 1])
                else:
                    nc.vector.tensor_copy(st_q[C:128, s, :], zsc[C:128, 0:C])
                    nc.vector.tensor_copy(st_kv[C:128, s, :], zsc[C:128, 0:128])

            # stage x via 4 chunked loads so the first transpose starts early
            xstage = big.tile([TCH, NTCH, C], F32)
            xsrc = x_d[:].rearrange("(r p) c -> p r c", p=TCH)
            for dc in range(4):
                nc.sync.dma_start(xstage[:, dc * 7:(dc + 1) * 7, :],
                                  xsrc[:, dc * 7:(dc + 1) * 7, :])

            # padded transposed image; rows 64-127 = shifted one spatial row
            xpT = big.tile([128, HP, WP], F32R)
            # [unshifted image; image shifted one column]
            xp3 = big.tile([128, HP, WP], F32R)
            qT = big.tile([C, N], F32R)            # q^T  [c, token]
            kT = big.tile([C, NPAD], F32R)         # k^T  [c, token], zero pad
            vT = big.tile([128, N], BF16)          # v''^T on partitions 64-127
            v_nat = big.tile([128, NKC, C + 1], BF16)  # [tok%128, chunk, e|1]
            b1 = big.tile([128, 1], F32)           # b' = bv@Wp + bp (parts 64+)
            b1sb = big.tile([C, 1], F32)

            nc.vector.tensor_copy(xpT[0:C, 0, :], zsc[0:C, 0:WP])
            nc.vector.tensor_copy(xpT[0:C, HP - 1, :], zsc[0:C, 0:WP])
            nc.vector.tensor_copy(xpT[0:C, :, 0:1], zsc[0:C, 0:HP].unsqueeze(2))
            nc.vector.tensor_copy(xpT[0:C, :, WP - 1:WP], zsc[0:C, 0:HP].unsqueeze(2))
            nc.vector.tensor_copy(xp3[C:128, :, WP - 1:WP], zsc[C:128, 0:HP].unsqueeze(2))
            nc.vector.tensor_copy(xp3[C:128, 0, :], zsc[C:128, 0:WP])
            nc.vector.tensor_copy(kT[:, N:NPAD], zsc[0:C, 0:NPAD - N])
            nc.vector.tensor_copy(v_nat[:, :, C], ones_f[:])

            with tc.tile_pool(name="ps2", bufs=1, space="PSUM") as ps2, \
                 tc.tile_pool(name="psT", bufs=1, space="PSUM") as psT, \
                 tc.tile_pool(name="psSa", bufs=1, space="PSUM") as psSa, \
                 tc.tile_pool(name="psSb", bufs=1, space="PSUM") as psSb, \
                 tc.tile_pool(name="psC", bufs=1, space="PSUM") as psC, \
                 tc.tile_pool(name="sbP", bufs=2 * NGRP) as sbP, \
                 tc.tile_pool(name="sbO", bufs=4) as sbO, \
                 tc.tile_pool(name="sbI", bufs=4) as sbI:

                # b' = bv @ Wp + bp (bf16 matmul at partitions 0-63, then a
                # partition-shifting SBUF->SBUF DMA up to 64-127)
                pb = psC.tile([C, 1], F32, name="pb", tag="ctx")
                nc.tensor.matmul(pb[:], Wp_b[:], bv_b[:], start=True, stop=True)
                nc.vector.tensor_tensor(b1sb[:], pb[:], bpT[:],
                                        mybir.AluOpType.add)
                nc.sync.dma_start(b1[C:128, :], b1sb[:])

                # ---- incremental emitters ----------------------------------
                st = {"tp": 0, "kv": 0, "vn": 0, "pctx": None}

                def emit_shift_dmas(rb):
                    # after chunk 7*rb+6, ship bulk shifted copies for dst
                    # rows 14rb..14rb+13 (src rows +1)
                    r0, r1 = 14 * rb, 14 * rb + 14
                    nc.sync.dma_start(xpT[C:128, r0:r1, :],
                                      xpT[0:C, r0 + 1:r1 + 1, :])
                    nc.gpsimd.dma_start(xp3[0:C, r0 + 1:r1 + 1, :],
                                        xpT[0:C, r0 + 1:r1 + 1, :])
                    nc.gpsimd.dma_start(xp3[C:128, r0 + 1:r1 + 1, 0:WP - 1],
                                        xpT[0:C, r0 + 1:r1 + 1, 1:WP])

                def emit_transposes(upto):
                    # x -> xpT rows 0-63, 2 spatial rows per PE transpose;
                    # early chunks may use ACT (it idles before the first
                    # exp); later ones stay on DVE (gpsimd can't read PSUM)
                    while st["tp"] < min(upto, NTCH):
                        r = st["tp"]
                        pt = psT.tile([C, TCH], F32, name="pt", tag="tp")
                        nc.tensor.transpose(pt[:], xstage[:, r, :],
                                            ident_f[0:TCH, 0:TCH])
                        dst = xpT[0:C, 1 + 2 * r:3 + 2 * r, 1:1 + W]
                        src = pt[:].rearrange("c (h w) -> c h w", w=W)
                        if r < 7 and r % 2 == 1:
                            nc.scalar.copy(dst, src)
                        else:
                            nc.vector.tensor_copy(dst, src)
                        st["tp"] += 1
                        if r % 7 == 6:
                            emit_shift_dmas(r // 7)
                        if r == NTCH - 1:
                            # tails: dst row 56 (row-shift) / row 57 (xp3)
                            nc.sync.dma_start(xpT[C:128, H:H + 1, :],
                                              xpT[0:C, HP - 1:HP, :])
                            nc.gpsimd.dma_start(xp3[0:C, HP - 1:HP, :],
                                                xpT[0:C, HP - 1:HP, :])
                            nc.gpsimd.dma_start(xp3[C:128, HP - 1:HP, 0:WP - 1],
                                                xpT[0:C, HP - 1:HP, 1:WP])

                def conv_matmuls(pdst, lhsT, ct, mwid):
                    r0 = ct * 8
                    for s in range(5):
                        if s < 3:
                            rhs = xpT[:, r0:r0 + 8, s:s + W]
                        elif s == 3:
                            rhs = xp3[:, r0 + 2:r0 + 10, 0:W]
                        else:
                            rhs = xpT[0:C, r0 + 2:r0 + 10, 2:2 + W]
                        nc.tensor.matmul(
                            pdst[:], lhsT[0:128 if s < 4 else C, s, 0:mwid],
                            rhs, start=(s == 0), stop=(s == 4))

                def emit_kv(upto):
                    while st["kv"] <= min(upto, NQT - 1):
                        ct = st["kv"]
                        emit_transposes(KV_CHUNK_NEED[ct])
                        pkv = ps2.tile([128, NQ], F32, name="pkv", tag="cv")
                        conv_matmuls(pkv, st_kv, ct, 128)
                        nc.vector.tensor_copy(kT[:, ct * NQ:(ct + 1) * NQ],
                                              pkv[0:C, :])
                        nc.vector.tensor_scalar_add(
                            vT[C:128, ct * NQ:(ct + 1) * NQ], pkv[C:128, :],
                            b1[C:128, 0:1])
                        st["kv"] += 1
                        # v_nat transposes, batched 4 chunks per PSUM tile /
                        # copy to amortize the DVE PSUM-access overhead
                        top = st["kv"] * NQ
                        while st["vn"] < NKC:
                            kc0 = st["vn"]
                            nb = min(4, NKC - kc0)
                            end = kc0 + nb - 1
                            cw_last = min(KC, N - end * KC)
                            if end * KC + cw_last > top:
                                break
                            tp = psC.tile([128, 4, C], BF16, name="tpv",
                                          tag="ctx")
                            for j in range(nb):
                                kc = kc0 + j
                                cw = min(KC, N - kc * KC)
                                nc.tensor.transpose(
                                    tp[0:cw, j, :],
                                    vT[C:128, kc * KC:kc * KC + cw],
                                    ident_b[C:128, C:128])
                            cw = min(KC, N - (kc0 + nb - 1) * KC)
                            if nb == 4 and cw == KC:
                                nc.vector.tensor_copy(
                                    v_nat[:, kc0:kc0 + nb, 0:C], tp[:, 0:nb, :])
                            else:
                                for j in range(nb):
                                    kc = kc0 + j
                                    cw = min(KC, N - kc * KC)
                                    nc.vector.tensor_copy(
                                        v_nat[0:cw, kc, 0:C], tp[0:cw, j, :])
                            st["vn"] += nb

                def emit_qconv_slots(pq, qt, slots):
                    r0 = qt * 8
                    for s in slots:
                        if s < 3:
                            rhs = xpT[:, r0:r0 + 8, s:s + W]
                        elif s == 3:
                            rhs = xp3[:, r0 + 2:r0 + 10, 0:W]
                        else:
                            rhs = xpT[0:C, r0 + 2:r0 + 10, 2:2 + W]
                        nc.tensor.matmul(
                            pq[:], st_q[0:128 if s < 4 else C, s, :],
                            rhs, start=(s == 0), stop=(s == 4))

                def emit_qcopy(pq, qt):
                    nc.vector.tensor_scalar_add(
                        qT[:, qt * NQ:(qt + 1) * NQ], pq[:], bqT[:, 0:1])

                def emit_av_batch(pT_tiles, s, half):
                    # 25 chunk-matmuls of one q-subtile, split in two halves;
                    # one pending psum group at a time (zero-region rule)
                    s0 = s * 128
                    sw = min(128, NQ - s0)
                    if half == 0:
                        st["pctx"] = psC.tile([128, C + 1], F32,
                                              name="pctx", tag="ctx")
                    pctx = st["pctx"]
                    chunks = range(0, 13) if half == 0 else range(13, NKC)
                    for kc in chunks:
                        g = next(i for i in range(NGRP)
                                 if GRP_OFF[i] <= kc < GRP_OFF[i] + GRP_SIZES[i])
                        j = kc - GRP_OFF[g]
                        cw = 64 if kc == NKC - 1 else 128
                        nc.tensor.matmul(
                            pctx[0:sw, :],
                            pT_tiles[g][0:cw, j, s0:s0 + sw],
                            v_nat[0:cw, kc, :],
                            start=(kc == 0), stop=(kc == NKC - 1))

                def emit_norm_sub(qt, s):
                    pctx = st["pctx"]
                    s0 = s * 128
                    sw = min(128, NQ - s0)
                    inv = sbI.tile([128, 1], F32, name="inv", tag="inv")
                    nc.vector.reciprocal(inv[0:sw, :], pctx[0:sw, C:C + 1])
                    osb = sbO.tile([128, E], F32, name="osb", tag="out")
                    nc.vector.tensor_scalar_mul(
                        osb[0:sw, :], pctx[0:sw, 0:C], inv[0:sw, 0:1])
                    nc.sync.dma_start(
                        out_d[qt * NQ + s0:qt * NQ + s0 + sw, :], osb[0:sw, :])

                def flush_prev(prev, g):
                    if prev is None or g >= 8:
                        return
                    qt_prev, pT_tiles = prev
                    emit_av_batch(pT_tiles, g // 2, g % 2)
                    if g % 2 == 1:
                        emit_norm_sub(qt_prev, g // 2)

                # ---- lead-in: q-conv(0) + kv(0) ----------------------------
                tap_sched = {0: (0,), 1: (1,), 2: (2,), 3: (3,), 4: (4,)}
                if level >= 2:
                    emit_transposes(KV_CHUNK_NEED[0])
                    pq = ps2.tile([C, NQ], F32, name="pq", tag="cv")
                    emit_qconv_slots(pq, 0, range(5))
                    emit_qcopy(pq, 0)
                    emit_kv(0)

                prev = None
                for qt in range(NQT if level >= 5 else 0):
                    q0 = qt * NQ
                    pq_next = None
                    pT_tiles = []
                    for g in range(NGRP):
                        gsz = GRP_SIZES[g]
                        pool = psSa if g % 2 == 0 else psSb
                        ps_s = pool.tile([128, gsz, 512], F32, name="ps_s",
                                         tag="sa" if g % 2 == 0 else "sb")
                        for j in range(gsz):
                            kc = GRP_OFF[g] + j
                            nc.tensor.matmul(
                                ps_s[:, j, 0:NQ],
                                kT[:, kc * KC:(kc + 1) * KC],
                                qT[:, q0:q0 + NQ],
                                start=True, stop=True)
                        flush_prev(prev, g)
                        if qt == 0:
                            # kv-conv tiles + v_nat stream in under tile 0
                            if g + 1 < NGRP:
                                emit_kv(KV_NEED[g + 1])
                            elif qt + 1 < NQT:
                                pq_next = ps2.tile([C, NQ], F32,
                                                   name="pq", tag="cv")
                                emit_qconv_slots(pq_next, 1, range(5))
                                emit_qcopy(pq_next, 1)
                        elif qt + 1 < NQT:
                            if g in tap_sched:
                                if pq_next is None:
                                    pq_next = ps2.tile([C, NQ], F32,
                                                       name="pq", tag="cv")
                                emit_qconv_slots(pq_next, qt + 1, tap_sched[g])
                            if g == 5:
                                emit_qcopy(pq_next, qt + 1)
                        pTt = sbP.tile([128, 3, NQ], BF16, name="pTt", tag="p")
                        nc.scalar.activation(
                            pTt[:, 0:gsz, :], ps_s[:, 0:gsz, 0:NQ],
                            AF.Exp, scale=8.0)
                        pT_tiles.append(pTt)
                    if level >= 6:
                        prev = (qt, pT_tiles)

                if prev is not None:
                    for g in range(8):
                        flush_prev(prev, g)

    nc.compile()
    return nc


def _get_nc():
    if "nc" not in _CACHE:
        _CACHE["nc"] = _build()
    return _CACHE["nc"]


def kernel(x, wq, bq, wk, bk, wv, bv, Wp, bp):
    from concourse.bass_utils import run_bass_kernel_spmd

    nc = _get_nc()
    x = np.ascontiguousarray(np.asarray(x, dtype=np.float32))
    shared = {
        "wq": np.ascontiguousarray(np.asarray(wq, np.float32).reshape(9, C)),
        "bq": np.ascontiguousarray(np.asarray(bq, np.float32)),
        "wk": np.ascontiguousarray(np.asarray(wk, np.float32).reshape(9, C)),
        "wv": np.ascontiguousarray(np.asarray(wv, np.float32).reshape(9, C)),
        "bv": np.ascontiguousarray(np.asarray(bv, np.float32)),
        "Wp": np.ascontiguousarray(np.asarray(Wp, np.float32)),
        "bp": np.ascontiguousarray(np.asarray(bp, np.float32)),
    }
    in_maps = [dict(shared, x=x[i].reshape(N, C)) for i in range(NCORES)]
    res = run_bass_kernel_spmd(nc, in_maps, core_ids=list(range(NCORES)))
    out = np.stack([res.results[i]["out"].reshape(H, W, E) for i in range(NCORES)])
    return out


# revision 25
# speedup vs baseline: 1.1193x; 1.0084x over previous
"""ConvAttention Trainium2 kernel (v4).

Per-core (data-parallel over batch, 8 cores, 1 image each):
  q/k/v = depthwise 3x3 conv over x [56,56,64], then full attention over
  N=3136 tokens with softmax(q.k * 8), then ctx @ Wp + bp.

Layout strategy:
  - Wp folded into the v-conv (lhsT blocks diag(wv_t) @ Wp): AV directly
    produces the projected output. bv/bp fold into b' = bv@Wp + bp added to
    v'' (exact via the rowsum trick); bk is dropped (constant along the
    softmax axis -> cancels exactly).
  - Convs are tap-stacked K=128 matmuls: partitions 64-127 of the padded
    transposed image hold a one-row-shifted copy (xpT) / one-col-shifted
    copy (xp3), built by SBUF->SBUF DMAs, so taps (0,j)+(1,j) and
    (2,0)+(2,1) pair into single matmuls: 5 matmuls per conv tile vs 9.
  - AV is out[qtok<=128, e] with lhsT = p^T chunks: natural [token, embed]
    output, no final transposes, no projection matmul; normalization is a
    per-partition reciprocal+mul. p/v'' are bf16 (rel err ~3e-3); q,k stay
    f32r (bf16/fp8 scores fail the 2e-2 gate via the x8 logit scale).
  - exp is the wall (~78us ACT busy): it runs from PSUM in alternating
    3-chunk/2-chunk groups (score pools of 3+2 banks) to amortize ACT
    access overhead while double-buffering QK against exp.
  - Everything else hides under exp: kv-convs + v_nat transposes interleave
    into attention tile 0's groups, the next tile's q-conv slots spread one
    per group, AV of tile t-1 flushes in half-sub batches between tile t's
    QK groups, and PSUM accumulation groups each keep an exclusive 2KB
    zero region (psSa 3 + psSb 2 + ps2 1 + psT 1 + psC 1 = 8 banks).
"""

import sys

import numpy as np

if "/opt/trn_rl_repo" not in sys.path:
    sys.path.insert(0, "/opt/trn_rl_repo")

H = 56
W = 56
C = 64
E = 64
N = H * W               # 3136 tokens
HP = H + 2              # padded
WP = W + 2
NQ = 448                # q-tile (8 spatial rows)
NQT = N // NQ           # 7
KC = 128                # k-chunk (partition dim of s^T tiles)
NKC = (N + KC - 1) // KC  # 25 (last chunk is 64 real tokens)
NPAD = NKC * KC         # 3200 (k padded with zeros)
TCH = 112               # x-transpose chunk = 2 spatial rows
NTCH = N // TCH         # 28
NCORES = 8

# exp chunk-groups per tile: alternating 3/2 so the two score pools fit in
# 5 PSUM banks total while still double-buffering QK against exp
GRP_SIZES = [3, 2, 3, 2, 3, 2, 3, 2, 3, 2]
GRP_OFF = [0, 3, 5, 8, 10, 13, 15, 18, 20, 23]
NGRP = len(GRP_SIZES)
# kv-conv tile that must be complete before QK of group g (any q-tile)
KV_NEED = [min(((GRP_OFF[g] + GRP_SIZES[g]) * KC - 1) // NQ, NQT - 1)
           for g in range(NGRP)]
# stacked conv slots: (lower tap, upper tap or None); taps t = 3*i + j.
# Slots 0-2 pair rows 0+1 via the row-shifted upper half of xpT; slots 3-5
# are single K=64 taps of row 2 (no column-shifted copy needed).
CONV_SLOTS = [(0, 3), (1, 4), (2, 5), (6, None), (7, None), (8, None)]
# row-shift bulk DMAs: (dst_row0, dst_row1, dep transpose chunk)
SHIFT_BULKS = [(0, 7, 3)] + [(7 * i + 1, 7 * i + 7, min((7 * i + 8) // 2, 27))
                             for i in range(1, 8)]
_BULK_BY_DEP = {}
for _i, (_d0, _d1, _dep) in enumerate(SHIFT_BULKS):
    _BULK_BY_DEP.setdefault(_dep, []).append(_i)
# x-transpose chunk count needed before kv-conv tile ct can run (covers the
# bulk-DMA dep for upper rows <= 8ct+7 and plain rows <= 8ct+9)
KV_CHUNK_NEED = []
for _ct in range(NQT):
    _bi = next(_i for _i, (_d0, _d1, _dep) in enumerate(SHIFT_BULKS)
               if _d1 >= 8 * _ct + 7)
    KV_CHUNK_NEED.append(max(SHIFT_BULKS[_bi][2] + 1, 4 * _ct + 5))

_CACHE = {}


def _build(level=99):
    import concourse.bacc as bacc
    import concourse.tile as tile
    from concourse import mybir
    from concourse.masks import make_identity

    F32 = mybir.dt.float32
    F32R = mybir.dt.float32r
    BF16 = mybir.dt.bfloat16
    AF = mybir.ActivationFunctionType

    nc = bacc.Bacc(None, target_bir_lowering=False, debug=False)

    x_d = nc.dram_tensor("x", [N, C], F32, kind="ExternalInput")
    wq_d = nc.dram_tensor("wq", [9, C], F32, kind="ExternalInput")
    bq_d = nc.dram_tensor("bq", [C], F32, kind="ExternalInput")
    wk_d = nc.dram_tensor("wk", [9, C], F32, kind="ExternalInput")
    wv_d = nc.dram_tensor("wv", [9, C], F32, kind="ExternalInput")
    bv_d = nc.dram_tensor("bv", [C], F32, kind="ExternalInput")
    Wp_d = nc.dram_tensor("Wp", [C, E], F32, kind="ExternalInput")
    bp_d = nc.dram_tensor("bp", [E], F32, kind="ExternalInput")
    out_d = nc.dram_tensor("out", [N, E], F32, kind="ExternalOutput")

    with tile.TileContext(nc) as tc:
        with tc.tile_pool(name="const", bufs=1) as const, \
             tc.tile_pool(name="big", bufs=1) as big:
            ident_f = const.tile([128, 128], F32)
            make_identity(nc, ident_f[:])
            ident = const.tile([128, 128], F32R)
            nc.vector.tensor_copy(ident[:], ident_f[:])
            ident_b = const.tile([128, 128], BF16)
            nc.vector.tensor_copy(ident_b[:], ident_f[:])
            zsc = const.tile([128, 128], F32)
            nc.vector.memset(zsc[:], 0.0)
            ones_f = const.tile([128, NKC], F32)
            nc.vector.memset(ones_f[:], 1.0)

            # per-channel conv weights, replicated on both partition halves
            # (the upper half feeds the tap-stacked lhsT rows 64-127)
            wqT = const.tile([128, 9], F32)
            wkT = const.tile([128, 9], F32)
            wvT = const.tile([128, 9], F32)
            for wt, wd in ((wqT, wq_d), (wkT, wk_d), (wvT, wv_d)):
                nc.sync.dma_start(wt[0:C, :], wd[:].transpose([1, 0]))
                nc.sync.dma_start(wt[C:128, :], wd[:].transpose([1, 0]))
            bqT = const.tile([C, 1], F32)
            nc.sync.dma_start(bqT[:], bq_d[:].unsqueeze(1))
            bvT = const.tile([C, 1], F32)
            nc.gpsimd.dma_start(bvT[:], bv_d[:].unsqueeze(1))
            bpT = const.tile([C, 1], F32)
            nc.gpsimd.dma_start(bpT[:], bp_d[:].unsqueeze(1))
            Wp_f = const.tile([128, E], F32)
            nc.gpsimd.dma_start(Wp_f[0:C, :], Wp_d[:])
            nc.gpsimd.dma_start(Wp_f[C:128, :], Wp_d[:])
            Wp_r = const.tile([128, E], F32R)
            nc.vector.tensor_copy(Wp_r[:], Wp_f[:])
            Wp_b = const.tile([C, E], BF16)
            nc.vector.tensor_copy(Wp_b[:], Wp_f[0:C, :])
            bv_b = const.tile([C, 1], BF16)
            nc.vector.tensor_copy(bv_b[:], bvT[:])

            # tap-stacked conv lhsT: st_q [128, 6, 64] (diag(wq)), st_kv
            # [128, 6, 128] (cols 0-63 diag(wk), 64-127 diag(wv) @ Wp);
            # upper halves of single-tap slots are never read
            st_q = const.tile([128, 6, C], F32R)
            st_kv = const.tile([128, 6, 128], F32R)
            idlo = ident[0:C, 0:C]
            idhi = ident[C:128, C:128]
            for s, (lt, ut) in enumerate(CONV_SLOTS):
                nc.vector.tensor_scalar_mul(st_q[0:C, s, :], idlo, wqT[0:C, lt:lt + 1])
                nc.vector.tensor_scalar_mul(st_kv[0:C, s, 0:C], idlo, wkT[0:C, lt:lt + 1])
                nc.vector.tensor_scalar_mul(st_kv[0:C, s, C:128], Wp_r[0:C, :], wvT[0:C, lt:lt + 1])
                if ut is not None:
                    nc.vector.tensor_scalar_mul(st_q[C:128, s, :], idhi, wqT[C:128, ut:ut + 1])
                    nc.vector.tensor_scalar_mul(st_kv[C:128, s, 0:C], idhi, wkT[C:128, ut:ut + 1])
                    nc.vector.tensor_scalar_mul(st_kv[C:128, s, C:128], Wp_r[C:128, :], wvT[C:128, ut:ut + 1])

            # stage x via 4 chunked loads so the first transpose starts early
            xstage = big.tile([TCH, NTCH, C], F32)
            xsrc = x_d[:].rearrange("(r p) c -> p r c", p=TCH)
            for dc in range(4):
                nc.sync.dma_start(xstage[:, dc * 7:(dc + 1) * 7, :],
                                  xsrc[:, dc * 7:(dc + 1) * 7, :])

            # padded transposed image; rows 64-127 = shifted one spatial row
            xpT = big.tile([128, HP, WP], F32R)
            qT = big.tile([C, N], F32R)            # q^T  [c, token]
            kT = big.tile([C, NPAD], F32R)         # k^T  [c, token], zero pad
            vT = big.tile([128, N], BF16)          # v''^T on partitions 64-127
            v_nat = big.tile([128, NKC, C + 1], BF16)  # [tok%128, chunk, e|1]
            b1 = big.tile([128, 1], F32)           # b' = bv@Wp + bp (parts 64+)
            b1sb = big.tile([C, 1], F32)

            nc.vector.tensor_copy(xpT[0:C, 0, :], zsc[0:C, 0:WP])
            nc.vector.tensor_copy(xpT[0:C, HP - 1, :], zsc[0:C, 0:WP])
            nc.vector.tensor_copy(xpT[0:C, :, 0:1], zsc[0:C, 0:HP].unsqueeze(2))
            nc.vector.tensor_copy(xpT[0:C, :, WP - 1:WP], zsc[0:C, 0:HP].unsqueeze(2))
            nc.vector.tensor_copy(kT[:, N:NPAD], zsc[0:C, 0:NPAD - N])
            nc.vector.tensor_copy(v_nat[:, :, C], ones_f[:])

            with tc.tile_pool(name="ps2", bufs=1, space="PSUM") as ps2, \
                 tc.tile_pool(name="psT", bufs=1, space="PSUM") as psT, \
                 tc.tile_pool(name="psSa", bufs=1, space="PSUM") as psSa, \
                 tc.tile_pool(name="psSb", bufs=1, space="PSUM") as psSb, \
                 tc.tile_pool(name="psC", bufs=1, space="PSUM") as psC, \
                 tc.tile_pool(name="sbP", bufs=2 * NGRP) as sbP, \
                 tc.tile_pool(name="sbO", bufs=4) as sbO, \
                 tc.tile_pool(name="sbI", bufs=4) as sbI:

                # b' = bv @ Wp + bp (bf16 matmul at partitions 0-63, then a
                # partition-shifting SBUF->SBUF DMA up to 64-127)
                pb = psC.tile([C, 1], F32, name="pb", tag="ctx")
                nc.tensor.matmul(pb[:], Wp_b[:], bv_b[:], start=True, stop=True)
                nc.vector.tensor_tensor(b1sb[:], pb[:], bpT[:],
                                        mybir.AluOpType.add)
                nc.sync.dma_start(b1[C:128, :], b1sb[:])

                # ---- incremental emitters ----------------------------------
                st = {"tp": 0, "kv": 0, "vn": 0, "pctx": None}

                # PE warmup: ramp the pstate clock during the x DMA wait
                ptw = psT.tile([C, TCH], F32, name="ptw", tag="tp")
                for _ in range(24):
                    nc.tensor.matmul(ptw[:], ident_b[:, 0:C],
                                     ident_b[:, 0:TCH], start=True, stop=True)

                def emit_transposes(upto):
                    # x -> xpT rows 0-63, 2 spatial rows per PE transpose;
                    # early chunks may use ACT (it idles before the first
                    # exp); later ones stay on DVE (gpsimd can't read PSUM).
                    # Row-shifted upper halves ship via HWDGE bulk DMAs as
                    # soon as their source chunks land.
                    while st["tp"] < min(upto, NTCH):
                        r = st["tp"]
                        pt = psT.tile([C, TCH], F32, name="pt", tag="tp")
                        nc.tensor.transpose(pt[:], xstage[:, r, :],
                                            ident_f[0:TCH, 0:TCH])
                        dst = xpT[0:C, 1 + 2 * r:3 + 2 * r, 1:1 + W]
                        src = pt[:].rearrange("c (h w) -> c h w", w=W)
                        if r < 7 and r % 2 == 1:
                            nc.scalar.copy(dst, src)
                        else:
                            nc.vector.tensor_copy(dst, src)
                        st["tp"] += 1
                        for bi in _BULK_BY_DEP.get(r, ()):
                            d0, d1, _dep = SHIFT_BULKS[bi]
                            nc.sync.dma_start(xpT[C:128, d0:d1 + 1, :],
                                              xpT[0:C, d0 + 1:d1 + 2, :])

                def conv_matmuls(pdst, lhsT, ct, mwid):
                    r0 = ct * 8
                    for s in range(6):
                        if s < 3:
                            rhs = xpT[:, r0:r0 + 8, s:s + W]
                            lh = lhsT[:, s, 0:mwid]
                        else:
                            rhs = xpT[0:C, r0 + 2:r0 + 10, (s - 3):(s - 3) + W]
                            lh = lhsT[0:C, s, 0:mwid]
                        nc.tensor.matmul(pdst[:], lh, rhs,
                                         start=(s == 0), stop=(s == 5))

                def emit_kv(upto):
                    while st["kv"] <= min(upto, NQT - 1):
                        ct = st["kv"]
                        emit_transposes(KV_CHUNK_NEED[ct])
                        pkv = ps2.tile([128, NQ], F32, name="pkv", tag="cv")
                        conv_matmuls(pkv, st_kv, ct, 128)
                        # split kT copies DVE/ACT to balance tile-0 load
                        if ct % 2 == 1:
                            nc.scalar.copy(kT[:, ct * NQ:(ct + 1) * NQ],
                                           pkv[0:C, :])
                        else:
                            nc.vector.tensor_copy(kT[:, ct * NQ:(ct + 1) * NQ],
                                                  pkv[0:C, :])
                        nc.vector.tensor_scalar_add(
                            vT[C:128, ct * NQ:(ct + 1) * NQ], pkv[C:128, :],
                            b1[C:128, 0:1])
                        st["kv"] += 1
                        # v_nat transposes, batched 4 chunks per PSUM tile /
                        # copy to amortize the DVE PSUM-access overhead
                        top = st["kv"] * NQ
                        while st["vn"] < NKC:
                            kc0 = st["vn"]
                            nb = min(4, NKC - kc0)
                            end = kc0 + nb - 1
                            cw_last = min(KC, N - end * KC)
                            if end * KC + cw_last > top:
                                break
                            tp = psC.tile([128, 4, C], BF16, name="tpv",
                                          tag="ctx")
                            for j in range(nb):
                                kc = kc0 + j
                                cw = min(KC, N - kc * KC)
                                nc.tensor.transpose(
                                    tp[0:cw, j, :],
                                    vT[C:128, kc * KC:kc * KC + cw],
                                    ident_b[C:128, C:128])
                            cw = min(KC, N - (kc0 + nb - 1) * KC)
                            if nb == 4 and cw == KC:
                                nc.vector.tensor_copy(
                                    v_nat[:, kc0:kc0 + nb, 0:C], tp[:, 0:nb, :])
                            else:
                                for j in range(nb):
                                    kc = kc0 + j
                                    cw = min(KC, N - kc * KC)
                                    nc.vector.tensor_copy(
                                        v_nat[0:cw, kc, 0:C], tp[0:cw, j, :])
                            st["vn"] += nb

                def emit_qconv_slots(pq, qt, slots):
                    r0 = qt * 8
                    for s in slots:
                        if s < 3:
                            rhs = xpT[:, r0:r0 + 8, s:s + W]
                            lh = st_q[:, s, :]
                        else:
                            rhs = xpT[0:C, r0 + 2:r0 + 10, (s - 3):(s - 3) + W]
                            lh = st_q[0:C, s, :]
                        nc.tensor.matmul(pq[:], lh, rhs,
                                         start=(s == 0), stop=(s == 5))

                def emit_qcopy(pq, qt):
                    nc.vector.tensor_scalar_add(
                        qT[:, qt * NQ:(qt + 1) * NQ], pq[:], bqT[:, 0:1])

                def emit_av_batch(pT_tiles, s, half):
                    # 25 chunk-matmuls of one q-subtile, split in two halves;
                    # one pending psum group at a time (zero-region rule)
                    s0 = s * 128
                    sw = min(128, NQ - s0)
                    if half == 0:
                        st["pctx"] = psC.tile([128, C + 1], F32,
                                              name="pctx", tag="ctx")
                    pctx = st["pctx"]
                    chunks = range(0, 13) if half == 0 else range(13, NKC)
                    for kc in chunks:
                        g = next(i for i in range(NGRP)
                                 if GRP_OFF[i] <= kc < GRP_OFF[i] + GRP_SIZES[i])
                        j = kc - GRP_OFF[g]
                        cw = 64 if kc == NKC - 1 else 128
                        nc.tensor.matmul(
                            pctx[0:sw, :],
                            pT_tiles[g][0:cw, j, s0:s0 + sw],
                            v_nat[0:cw, kc, :],
                            start=(kc == 0), stop=(kc == NKC - 1))

                def emit_norm_sub(qt, s):
                    pctx = st["pctx"]
                    s0 = s * 128
                    sw = min(128, NQ - s0)
                    inv = sbI.tile([128, 1], F32, name="inv", tag="inv")
                    nc.vector.reciprocal(inv[0:sw, :], pctx[0:sw, C:C + 1])
                    osb = sbO.tile([128, E], F32, name="osb", tag="out")
                    nc.vector.tensor_scalar_mul(
                        osb[0:sw, :], pctx[0:sw, 0:C], inv[0:sw, 0:1])
                    nc.sync.dma_start(
                        out_d[qt * NQ + s0:qt * NQ + s0 + sw, :], osb[0:sw, :])

                def flush_prev(prev, g):
                    # AV batches one group later than minimal so the psC WAR
                    # (sub start vs previous norm) never stalls the PE
                    if prev is None or g < 1 or g > 8:
                        return
                    qt_prev, pT_tiles = prev
                    emit_av_batch(pT_tiles, (g - 1) // 2, (g - 1) % 2)
                    if (g - 1) % 2 == 1:
                        emit_norm_sub(qt_prev, (g - 1) // 2)

                # ---- lead-in: q-conv(0) + kv(0) ----------------------------
                tap_sched = {0: (0,), 1: (1,), 2: (2,), 3: (3,), 4: (4,), 5: (5,)}
                if level >= 2:
                    emit_transposes(KV_CHUNK_NEED[0])
                    pq = ps2.tile([C, NQ], F32, name="pq", tag="cv")
                    emit_qconv_slots(pq, 0, range(6))
                    emit_qcopy(pq, 0)
                    emit_kv(0)

                prev = None
                for qt in range(NQT if level >= 5 else 0):
                    q0 = qt * NQ
                    pq_next = None
                    pT_tiles = []
                    for g in range(NGRP):
                        gsz = GRP_SIZES[g]
                        pool = psSa if g % 2 == 0 else psSb
                        ps_s = pool.tile([128, gsz, 512], F32, name="ps_s",
                                         tag="sa" if g % 2 == 0 else "sb")
                        for j in range(gsz):
                            kc = GRP_OFF[g] + j
                            nc.tensor.matmul(
                                ps_s[:, j, 0:NQ],
                                kT[:, kc * KC:(kc + 1) * KC],
                                qT[:, q0:q0 + NQ],
                                start=True, stop=True)
                        flush_prev(prev, g)
                        if qt == 0:
                            # kv-conv tiles + v_nat stream in under tile 0
                            if g + 1 < NGRP:
                                emit_kv(KV_NEED[g + 1])
                            elif qt + 1 < NQT:
                                pq_next = ps2.tile([C, NQ], F32,
                                                   name="pq", tag="cv")
                                emit_qconv_slots(pq_next, 1, range(6))
                                emit_qcopy(pq_next, 1)
                        elif qt + 1 < NQT:
                            if g in tap_sched:
                                if pq_next is None:
                                    pq_next = ps2.tile([C, NQ], F32,
                                                       name="pq", tag="cv")
                                emit_qconv_slots(pq_next, qt + 1, tap_sched[g])
                            if g == 6:
                                emit_qcopy(pq_next, qt + 1)
                        pTt = sbP.tile([128, 3, NQ], BF16, name="pTt", tag="p")
                        nc.scalar.activation(
                            pTt[:, 0:gsz, :], ps_s[:, 0:gsz, 0:NQ],
                            AF.Exp, scale=8.0)
                        pT_tiles.append(pTt)
                    if level >= 6:
                        prev = (qt, pT_tiles)

                if prev is not None:
                    for g in range(1, 9):
                        flush_prev(prev, g)

    nc.compile()
    return nc


def _get_nc():
    if "nc" not in _CACHE:
        _CACHE["nc"] = _build()
    return _CACHE["nc"]


def kernel(x, wq, bq, wk, bk, wv, bv, Wp, bp):
    from concourse.bass_utils import run_bass_kernel_spmd

    nc = _get_nc()
    x = np.ascontiguousarray(np.asarray(x, dtype=np.float32))
    shared = {
        "wq": np.ascontiguousarray(np.asarray(wq, np.float32).reshape(9, C)),
        "bq": np.ascontiguousarray(np.asarray(bq, np.float32)),
        "wk": np.ascontiguousarray(np.asarray(wk, np.float32).reshape(9, C)),
        "wv": np.ascontiguousarray(np.asarray(wv, np.float32).reshape(9, C)),
        "bv": np.ascontiguousarray(np.asarray(bv, np.float32)),
        "Wp": np.ascontiguousarray(np.asarray(Wp, np.float32)),
        "bp": np.ascontiguousarray(np.asarray(bp, np.float32)),
    }
    in_maps = [dict(shared, x=x[i].reshape(N, C)) for i in range(NCORES)]
    res = run_bass_kernel_spmd(nc, in_maps, core_ids=list(range(NCORES)))
    out = np.stack([res.results[i]["out"].reshape(H, W, E) for i in range(NCORES)])
    return out
